# revision 12
# baseline (speedup 1.0000x reference)
"""NT-Xent loss (B=4096, D=128, T=0.07) on 8 Trainium2 NeuronCores.

Strategy (one SPMD Bass program, 8 cores):
  - Host: z = concat(z_i, z_j) [8192,128], scale by 1/sqrt(T), transpose to
    zT [128, 8192], cast fp16 (PE runs fp16 at 4x the fp32 rate; validated
    loss rel-err ~1.4e-6).  Core c receives zT rotated left by c*1024 cols so
    its own 1024 rows sit at columns 0..1023 -> the self-sim diag block and
    the positive-pair diag block land at compile-time-constant offsets on
    every core (one uniform SPMD program).
  - Device, per 128-row tile t (8 tiles/core), ONE pass over the [128, 8192]
    similarity slab in eight [128,1024] PSUM chunks (4 PSUM buffers in flight so the
    fill->reduce->exp->release chain pipelines):
      PE   : 2 matmuls (N=512, fp16) per chunk; the self-diag block gets
             -1e5*I added via an extra accumulating matmul (identity trick),
             so it can never win the max and exp() flushes it to 0.
      DVE  : reduce_max(negate) straight from PSUM -> per-quarter -max m_q;
             scalar_tensor_tensor extracts the positive-pair diagonal.
      ACT  : activation(Exp, bias=-m_q, accum_out) straight from the same
             PSUM quarter = fused exp + row-sum with a PER-QUARTER shift.
             (DVE and ACT read the same quarter concurrently via separate
             PSUM ports; PE fills the other buffer meanwhile.)
      tail : quarter sums are rescaled exactly: stot = sum_q ssq_q*e^{m_q-m}
             (+ e^{pos-m} for the duplicated positive), loss = ln(stot)+m-pos.
             All tail ops are [128,4]/[128,1] sized.
  - Host: sum the 8 x [128,8] per-row losses, divide by 8192.

This avoids any second PE pass and any PSUM->SBUF evacuation of the slab:
each PSUM element is read exactly twice (once by DVE for the max, once by
ACT for the exp-sum), which is the minimum this algorithm needs.

The toolchain's walrus allows only ONE sync-wait per TPB instruction;
_split_waits() hoists extra waits onto injected NoOps post-Tile.
"""

import os
import numpy as np

N_CORES = 8
B = 4096
NROWS = 2 * B           # 8192
ROWS_PER_CORE = NROWS // N_CORES       # 1024
TILES_PER_CORE = ROWS_PER_CORE // 128  # 8
CHUNK = 1024
NCHUNK = 8192 // CHUNK  # 8
TEMP = 0.07
MASK_NEG = -1.0e5

_cached = {}


def _split_waits(nc, limit=1):
    import bass_rust
    import concourse.mybir as mybir

    n = 0
    for f in nc.m.functions:
        for blk in f.blocks:
            new_insts = []
            for inst in blk.instructions:
                si = inst.sync_info
                waits = list(si.on_wait) if (si and si.on_wait) else []
                if len(waits) > limit:
                    for w in waits[:-limit]:
                        nop = bass_rust.InstNoOp(name=f"waitnop-{n}")
                        n += 1
                        nop.engine = inst.engine
                        nop.sync_info = mybir.SyncInfo(on_wait=[w], on_update=[])
                        new_insts.append(nop)
                    inst.sync_info = mybir.SyncInfo(
                        on_wait=waits[-limit:], on_update=list(si.on_update or [])
                    )
                new_insts.append(inst)
            blk.instructions = new_insts


def _build_module():
    import concourse.bass as bass
    import concourse.mybir as mybir
    from concourse.tile import TileContext
    from contextlib import ExitStack

    f32 = mybir.dt.float32
    f16 = mybir.dt.float16
    Alu = mybir.AluOpType
    Act = mybir.ActivationFunctionType
    X = mybir.AxisListType.X

    nc = bass.Bass()

    zq_d = [
        nc.dram_tensor(f"zq{q}", [128, 2048], f16, kind="ExternalInput")
        for q in range(4)
    ]
    posi_d = nc.dram_tensor("posI", [128, 128], f32, kind="ExternalInput")
    mskb_d = nc.dram_tensor("mskB", [128, 128], f32, kind="ExternalInput")
    loss_d = nc.dram_tensor("loss", [128, TILES_PER_CORE], f32, kind="ExternalOutput")

    with ExitStack() as ctx:
        tc = ctx.enter_context(TileContext(nc))
        const = ctx.enter_context(tc.tile_pool(name="const", bufs=1))
        egp = ctx.enter_context(tc.tile_pool(name="egp", bufs=3))
        psum = ctx.enter_context(
            tc.tile_pool(name="psum", bufs=4, space=bass.MemorySpace.PSUM)
        )
        stats = ctx.enter_context(tc.tile_pool(name="stats", bufs=4))

        zqt = []
        for q in range(4):
            zt = const.tile([128, 2048], f16, tag=f"zq{q}")
            nc.gpsimd.dma_start(out=zt[:, 0:1024], in_=zq_d[q][:, 0:1024])
            nc.gpsimd.dma_start(out=zt[:, 1024:2048], in_=zq_d[q][:, 1024:2048])
            zqt.append(zt)
        posit = const.tile([128, 128], f32, tag="posI")
        nc.gpsimd.dma_start(out=posit, in_=posi_d[:])
        mskbt = const.tile([128, 128], f32, tag="mskB")
        nc.gpsimd.dma_start(out=mskbt, in_=mskb_d[:])
        losst = const.tile([128, TILES_PER_CORE], f32, tag="losst")

        def chunk_matmuls(P, t, e):
            # chunk e covers global cols [e*CHUNK, (e+1)*CHUNK)
            lhsT = zqt[0][:, t * 128 : (t + 1) * 128]
            dj = (t * 128) // 512  # 512-piece of chunk 0 containing self-diag
            for j in range(2):
                gcol = e * CHUNK + j * 512
                is_diag_chunk = e == 0 and j == dj
                nc.tensor.matmul(
                    P[:, j * 512 : (j + 1) * 512],
                    lhsT,
                    zqt[gcol // 2048][:, gcol % 2048 : gcol % 2048 + 512],
                    start=True,
                    stop=not is_diag_chunk,
                    skip_group_check=True,
                )
                if is_diag_chunk:
                    # self-diag block += -1e5*I  (I.T @ (-1e5*I) accumulated)
                    nc.tensor.matmul(
                        P[:, t * 128 : t * 128 + 128],
                        posit,
                        mskbt,
                        start=False,
                        stop=True,
                        skip_group_check=True,
                    )

        POS_E = 4096 // CHUNK  # chunk holding the positive-pair diagonal
        for t in range(TILES_PER_CORE):
            nm = stats.tile([128, NCHUNK], f32, tag="nm")   # -m_e per chunk
            ssq = stats.tile([128, NCHUNK], f32, tag="ssq")  # sum e^{x-m_e}
            pos = stats.tile([128, 1], f32, tag="pos")
            scr = stats.tile([128, 128], f32, tag="scr")

            for e in range(NCHUNK):
                P = psum.tile([128, CHUNK], f32, tag="P")
                chunk_matmuls(P, t, e)
                if e == POS_E:
                    # positive-pair diag at chunk-local cols [t*128, +128)
                    nc.vector.scalar_tensor_tensor(
                        out=scr,
                        in0=P[:, t * 128 : t * 128 + 128],
                        scalar=1.0,
                        in1=posit,
                        op0=Alu.mult,
                        op1=Alu.mult,
                        accum_out=pos,
                    )
                nc.vector.reduce_max(out=nm[:, e : e + 1], in_=P, axis=X, negate=True)
                eg = egp.tile([128, CHUNK], f32, tag="eg")
                nc.scalar.activation(
                    out=eg,
                    in_=P,
                    func=Act.Exp,
                    bias=nm[:, e : e + 1],
                    scale=1.0,
                    accum_out=ssq[:, e : e + 1],
                )

            # tail: exact recombination of the four quarter-shifted sums
            # m = global row max = -min_q nm_q  (mt := -m)
            mt = stats.tile([128, 1], f32, tag="mt")
            nc.vector.tensor_reduce(out=mt, in_=nm, axis=X, op=Alu.min)
            f = stats.tile([128, NCHUNK], f32, tag="f")
            nc.scalar.activation(out=f, in_=nm, func=Act.Exp, scale=-1.0, bias=mt)
            s03 = stats.tile([128, 1], f32, tag="s03")
            nc.vector.scalar_tensor_tensor(
                out=scr[:, 0:NCHUNK],
                in0=ssq,
                scalar=1.0,
                in1=f,
                op0=Alu.mult,
                op1=Alu.mult,
                accum_out=s03,
            )
            pe = stats.tile([128, 1], f32, tag="pe")   # e^{pos - m}
            nc.scalar.activation(out=pe, in_=pos, func=Act.Exp, bias=mt, scale=1.0)
            stot = stats.tile([128, 1], f32, tag="stot")
            nc.vector.tensor_add(stot, s03, pe)
            # loss = ln(stot) + m - pos = lg - mt - pos
            lg = stats.tile([128, 1], f32, tag="lg")
            nc.scalar.activation(out=lg, in_=stot, func=Act.Ln)
            lp = stats.tile([128, 1], f32, tag="lp")
            nc.vector.tensor_sub(lp, lg, mt)
            nc.vector.tensor_sub(losst[:, t : t + 1], lp, pos)

        nc.gpsimd.dma_start(out=loss_d[:], in_=losst)

    _split_waits(nc)
    return nc


def _get_module():
    if "nc" not in _cached:
        _cached["nc"] = _build_module()
    return _cached["nc"]


def _host_inputs(z_i, z_j):
    z = np.concatenate(
        [np.asarray(z_i, np.float32), np.asarray(z_j, np.float32)], axis=0
    )
    s = np.float32(1.0 / np.sqrt(TEMP))
    zT = np.ascontiguousarray((z * s).T).astype(np.float16)  # [128, 8192]

    posI = np.eye(128, dtype=np.float32)
    mskB = np.float32(MASK_NEG) * np.eye(128, dtype=np.float32)

    in_maps = []
    for c in range(N_CORES):
        k = c * ROWS_PER_CORE
        rot = np.concatenate([zT[:, k:], zT[:, :k]], axis=1)
        im = {
            f"zq{q}": np.ascontiguousarray(rot[:, q * 2048 : (q + 1) * 2048])
            for q in range(4)
        }
        im["posI"] = posI
        im["mskB"] = mskB
        in_maps.append(im)
    return in_maps


def run_full(z_i, z_j, trace=False, trace_kwargs=None):
    """Run on 8 cores; returns (loss_scalar, BassKernelResults)."""
    from concourse.bass_utils import run_bass_kernel_spmd

    nc = _get_module()
    in_maps = _host_inputs(z_i, z_j)
    res = run_bass_kernel_spmd(
        nc,
        in_maps,
        core_ids=list(range(N_CORES)),
        trace=trace,
        **(trace_kwargs or {}),
    )
    total = np.float64(0.0)
    for c in range(N_CORES):
        total += res.results[c]["loss"].astype(np.float64).sum()
    loss = np.array(total / NROWS, dtype=np.float32)
    return loss, res


def kernel(z_i, z_j):
    loss, _ = run_full(z_i, z_j, trace=bool(os.environ.get("KERNEL_TRACE")))
    return loss


# revision 13
# speedup vs baseline: 1.0094x; 1.0094x over previous
"""NT-Xent loss (B=4096, D=128, T=0.07) on 8 Trainium2 NeuronCores.

Strategy (one SPMD Bass program, 8 cores):
  - Host: z = concat(z_i, z_j) [8192,128], scale by 1/sqrt(T), transpose to
    zT [128, 8192], cast fp16 (PE runs fp16 at 4x the fp32 rate; validated
    loss rel-err ~1.4e-6).  Core c receives zT rotated left by c*1024 cols so
    its own 1024 rows sit at columns 0..1023 -> the self-sim diag block and
    the positive-pair diag block land at compile-time-constant offsets on
    every core (one uniform SPMD program).
  - Device, per 128-row tile t (8 tiles/core), ONE pass over the [128, 8192]
    similarity slab in eight [128,1024] PSUM chunks (4 PSUM buffers in flight so the
    fill->reduce->exp->release chain pipelines):
      PE   : 2 matmuls (N=512, fp16) per chunk; the self-diag block gets
             -1e5*I added via an extra accumulating matmul (identity trick),
             so it can never win the max and exp() flushes it to 0.
      DVE  : reduce_max(negate) straight from PSUM -> per-quarter -max m_q;
             scalar_tensor_tensor extracts the positive-pair diagonal.
      ACT  : activation(Exp, bias=-m_q, accum_out) straight from the same
             PSUM quarter = fused exp + row-sum with a PER-QUARTER shift.
             (DVE and ACT read the same quarter concurrently via separate
             PSUM ports; PE fills the other buffer meanwhile.)
      tail : quarter sums are rescaled exactly: stot = sum_q ssq_q*e^{m_q-m}
             (+ e^{pos-m} for the duplicated positive), loss = ln(stot)+m-pos.
             All tail ops are [128,4]/[128,1] sized.
  - Host: sum the 8 x [128,8] per-row losses, divide by 8192.

This avoids any second PE pass and any PSUM->SBUF evacuation of the slab:
each PSUM element is read exactly twice (once by DVE for the max, once by
ACT for the exp-sum), which is the minimum this algorithm needs.

The toolchain's walrus allows only ONE sync-wait per TPB instruction;
_split_waits() hoists extra waits onto injected NoOps post-Tile.
"""

import os
import numpy as np

N_CORES = 8
B = 4096
NROWS = 2 * B           # 8192
ROWS_PER_CORE = NROWS // N_CORES       # 1024
TILES_PER_CORE = ROWS_PER_CORE // 128  # 8
CHUNK = 1024
NCHUNK = 8192 // CHUNK  # 8
TEMP = 0.07
MASK_NEG = -1.0e5

_cached = {}


def _split_waits(nc, limit=1):
    import bass_rust
    import concourse.mybir as mybir

    n = 0
    for f in nc.m.functions:
        for blk in f.blocks:
            new_insts = []
            for inst in blk.instructions:
                si = inst.sync_info
                waits = list(si.on_wait) if (si and si.on_wait) else []
                if len(waits) > limit:
                    for w in waits[:-limit]:
                        nop = bass_rust.InstNoOp(name=f"waitnop-{n}")
                        n += 1
                        nop.engine = inst.engine
                        nop.sync_info = mybir.SyncInfo(on_wait=[w], on_update=[])
                        new_insts.append(nop)
                    inst.sync_info = mybir.SyncInfo(
                        on_wait=waits[-limit:], on_update=list(si.on_update or [])
                    )
                new_insts.append(inst)
            blk.instructions = new_insts


def _build_module():
    import concourse.bass as bass
    import concourse.mybir as mybir
    from concourse.tile import TileContext
    from contextlib import ExitStack

    f32 = mybir.dt.float32
    f16 = mybir.dt.float16
    Alu = mybir.AluOpType
    Act = mybir.ActivationFunctionType
    X = mybir.AxisListType.X

    nc = bass.Bass()

    zq_d = [
        nc.dram_tensor(f"zq{q}", [128, 2048], f16, kind="ExternalInput")
        for q in range(4)
    ]
    posi_d = nc.dram_tensor("posI", [128, 128], f32, kind="ExternalInput")
    mskb_d = nc.dram_tensor("mskB", [128, 128], f32, kind="ExternalInput")
    loss_d = nc.dram_tensor("loss", [128, TILES_PER_CORE], f32, kind="ExternalOutput")

    with ExitStack() as ctx:
        tc = ctx.enter_context(TileContext(nc))
        const = ctx.enter_context(tc.tile_pool(name="const", bufs=1))
        egp = ctx.enter_context(tc.tile_pool(name="egp", bufs=2))
        psum = ctx.enter_context(
            tc.tile_pool(name="psum", bufs=4, space=bass.MemorySpace.PSUM)
        )
        stats = ctx.enter_context(tc.tile_pool(name="stats", bufs=3))

        zqt = []
        for q in range(4):
            zt = const.tile([128, 2048], f16, tag=f"zq{q}")
            nc.gpsimd.dma_start(out=zt[:, 0:1024], in_=zq_d[q][:, 0:1024])
            nc.gpsimd.dma_start(out=zt[:, 1024:2048], in_=zq_d[q][:, 1024:2048])
            zqt.append(zt)
        posit = const.tile([128, 128], f32, tag="posI")
        nc.gpsimd.dma_start(out=posit, in_=posi_d[:])
        mskbt = const.tile([128, 128], f32, tag="mskB")
        nc.gpsimd.dma_start(out=mskbt, in_=mskb_d[:])
        losst = const.tile([128, TILES_PER_CORE], f32, tag="losst")

        def chunk_matmuls(P, t, e):
            # chunk e covers global cols [e*CHUNK, (e+1)*CHUNK)
            lhsT = zqt[0][:, t * 128 : (t + 1) * 128]
            dj = (t * 128) // 512  # 512-piece of chunk 0 containing self-diag
            for j in range(2):
                gcol = e * CHUNK + j * 512
                is_diag_chunk = e == 0 and j == dj
                nc.tensor.matmul(
                    P[:, j * 512 : (j + 1) * 512],
                    lhsT,
                    zqt[gcol // 2048][:, gcol % 2048 : gcol % 2048 + 512],
                    start=True,
                    stop=not is_diag_chunk,
                    skip_group_check=True,
                )
                if is_diag_chunk:
                    # self-diag block += -1e5*I  (I.T @ (-1e5*I) accumulated)
                    nc.tensor.matmul(
                        P[:, t * 128 : t * 128 + 128],
                        posit,
                        mskbt,
                        start=False,
                        stop=True,
                        skip_group_check=True,
                    )

        POS_E = 4096 // CHUNK  # chunk holding the positive-pair diagonal
        for t in range(TILES_PER_CORE):
            nm = stats.tile([128, NCHUNK], f32, tag="nm")   # -m_e per chunk
            ssq = stats.tile([128, NCHUNK], f32, tag="ssq")  # sum e^{x-m_e}
            pos = stats.tile([128, 1], f32, tag="pos")
            scr = stats.tile([128, 128], f32, tag="scr")

            for e in range(NCHUNK):
                P = psum.tile([128, CHUNK], f32, tag="P")
                chunk_matmuls(P, t, e)
                if e == POS_E:
                    # positive-pair diag at chunk-local cols [t*128, +128)
                    nc.vector.scalar_tensor_tensor(
                        out=scr,
                        in0=P[:, t * 128 : t * 128 + 128],
                        scalar=1.0,
                        in1=posit,
                        op0=Alu.mult,
                        op1=Alu.mult,
                        accum_out=pos,
                    )
                nc.vector.reduce_max(out=nm[:, e : e + 1], in_=P, axis=X, negate=True)
                eg = egp.tile([128, CHUNK], f32, tag="eg")
                nc.scalar.activation(
                    out=eg,
                    in_=P,
                    func=Act.Exp,
                    bias=nm[:, e : e + 1],
                    scale=1.0,
                    accum_out=ssq[:, e : e + 1],
                )

            # tail: exact recombination of the four quarter-shifted sums
            # m = global row max = -min_q nm_q  (mt := -m)
            mt = stats.tile([128, 1], f32, tag="mt")
            nc.vector.tensor_reduce(out=mt, in_=nm, axis=X, op=Alu.min)
            f = stats.tile([128, NCHUNK], f32, tag="f")
            nc.scalar.activation(out=f, in_=nm, func=Act.Exp, scale=-1.0, bias=mt)
            s03 = stats.tile([128, 1], f32, tag="s03")
            nc.vector.scalar_tensor_tensor(
                out=scr[:, 0:NCHUNK],
                in0=ssq,
                scalar=1.0,
                in1=f,
                op0=Alu.mult,
                op1=Alu.mult,
                accum_out=s03,
            )
            pe = stats.tile([128, 1], f32, tag="pe")   # e^{pos - m}
            nc.scalar.activation(out=pe, in_=pos, func=Act.Exp, bias=mt, scale=1.0)
            stot = stats.tile([128, 1], f32, tag="stot")
            nc.vector.tensor_add(stot, s03, pe)
            # loss = ln(stot) + m - pos = lg - mt - pos
            lg = stats.tile([128, 1], f32, tag="lg")
            nc.scalar.activation(out=lg, in_=stot, func=Act.Ln)
            lp = stats.tile([128, 1], f32, tag="lp")
            nc.vector.tensor_sub(lp, lg, mt)
            nc.vector.tensor_sub(losst[:, t : t + 1], lp, pos)

        nc.gpsimd.dma_start(out=loss_d[:], in_=losst)

    _split_waits(nc)
    return nc


def _get_module():
    if "nc" not in _cached:
        _cached["nc"] = _build_module()
    return _cached["nc"]


def _host_inputs(z_i, z_j):
    z = np.concatenate(
        [np.asarray(z_i, np.float32), np.asarray(z_j, np.float32)], axis=0
    )
    s = np.float32(1.0 / np.sqrt(TEMP))
    zT = np.ascontiguousarray((z * s).T).astype(np.float16)  # [128, 8192]

    posI = np.eye(128, dtype=np.float32)
    mskB = np.float32(MASK_NEG) * np.eye(128, dtype=np.float32)

    in_maps = []
    for c in range(N_CORES):
        k = c * ROWS_PER_CORE
        rot = np.concatenate([zT[:, k:], zT[:, :k]], axis=1)
        im = {
            f"zq{q}": np.ascontiguousarray(rot[:, q * 2048 : (q + 1) * 2048])
            for q in range(4)
        }
        im["posI"] = posI
        im["mskB"] = mskB
        in_maps.append(im)
    return in_maps


def run_full(z_i, z_j, trace=False, trace_kwargs=None):
    """Run on 8 cores; returns (loss_scalar, BassKernelResults)."""
    from concourse.bass_utils import run_bass_kernel_spmd

    nc = _get_module()
    in_maps = _host_inputs(z_i, z_j)
    res = run_bass_kernel_spmd(
        nc,
        in_maps,
        core_ids=list(range(N_CORES)),
        trace=trace,
        **(trace_kwargs or {}),
    )
    total = np.float64(0.0)
    for c in range(N_CORES):
        total += res.results[c]["loss"].astype(np.float64).sum()
    loss = np.array(total / NROWS, dtype=np.float32)
    return loss, res


def kernel(z_i, z_j):
    loss, _ = run_full(z_i, z_j, trace=bool(os.environ.get("KERNEL_TRACE")))
    return loss


# revision 14
# speedup vs baseline: 1.0156x; 1.0061x over previous
"""NT-Xent loss (B=4096, D=128, T=0.07) on 8 Trainium2 NeuronCores.

Strategy (one SPMD Bass program, 8 cores):
  - Host: z = concat(z_i, z_j) [8192,128], scale by 1/sqrt(T), transpose to
    zT [128, 8192], cast fp16 (PE runs fp16 at 4x the fp32 rate; validated
    loss rel-err ~1.4e-6).  Core c receives zT rotated left by c*1024 cols so
    its own 1024 rows sit at columns 0..1023 -> the self-sim diag block and
    the positive-pair diag block land at compile-time-constant offsets on
    every core (one uniform SPMD program).
  - Device, per 128-row tile t (8 tiles/core), ONE pass over the [128, 8192]
    similarity slab in eight [128,1024] PSUM chunks (4 PSUM buffers in flight so the
    fill->reduce->exp->release chain pipelines):
      PE   : 2 matmuls (N=512, fp16) per chunk; the self-diag block gets
             -1e5*I added via an extra accumulating matmul (identity trick),
             so it can never win the max and exp() flushes it to 0.
      DVE  : reduce_max(negate) straight from PSUM -> per-quarter -max m_q;
             scalar_tensor_tensor extracts the positive-pair diagonal.
      ACT  : activation(Exp, bias=-m_q, accum_out) straight from the same
             PSUM quarter = fused exp + row-sum with a PER-QUARTER shift.
             (DVE and ACT read the same quarter concurrently via separate
             PSUM ports; PE fills the other buffer meanwhile.)
      tail : quarter sums are rescaled exactly: stot = sum_q ssq_q*e^{m_q-m}
             (+ e^{pos-m} for the duplicated positive), loss = ln(stot)+m-pos.
             All tail ops are [128,4]/[128,1] sized.
  - Host: sum the 8 x [128,8] per-row losses, divide by 8192.

This avoids any second PE pass and any PSUM->SBUF evacuation of the slab:
each PSUM element is read exactly twice (once by DVE for the max, once by
ACT for the exp-sum), which is the minimum this algorithm needs.

The toolchain's walrus allows only ONE sync-wait per TPB instruction;
_split_waits() hoists extra waits onto injected NoOps post-Tile.
"""

import os
import numpy as np

N_CORES = 8
B = 4096
NROWS = 2 * B           # 8192
ROWS_PER_CORE = NROWS // N_CORES       # 1024
TILES_PER_CORE = ROWS_PER_CORE // 128  # 8
CHUNK = 1024
NCHUNK = 8192 // CHUNK  # 8
TEMP = 0.07
MASK_NEG = -1.0e5

_cached = {}


def _split_waits(nc, limit=1):
    import bass_rust
    import concourse.mybir as mybir

    n = 0
    for f in nc.m.functions:
        for blk in f.blocks:
            new_insts = []
            for inst in blk.instructions:
                si = inst.sync_info
                waits = list(si.on_wait) if (si and si.on_wait) else []
                if len(waits) > limit:
                    for w in waits[:-limit]:
                        nop = bass_rust.InstNoOp(name=f"waitnop-{n}")
                        n += 1
                        nop.engine = inst.engine
                        nop.sync_info = mybir.SyncInfo(on_wait=[w], on_update=[])
                        new_insts.append(nop)
                    inst.sync_info = mybir.SyncInfo(
                        on_wait=waits[-limit:], on_update=list(si.on_update or [])
                    )
                new_insts.append(inst)
            blk.instructions = new_insts


def _build_module():
    import concourse.bass as bass
    import concourse.mybir as mybir
    from concourse.tile import TileContext
    from contextlib import ExitStack

    f32 = mybir.dt.float32
    f16 = mybir.dt.float16
    Alu = mybir.AluOpType
    Act = mybir.ActivationFunctionType
    X = mybir.AxisListType.X

    nc = bass.Bass()

    zq_d = [
        nc.dram_tensor(f"zq{q}", [128, 2048], f16, kind="ExternalInput")
        for q in range(4)
    ]
    posi_d = nc.dram_tensor("posI", [128, 128], f32, kind="ExternalInput")
    mskb_d = nc.dram_tensor("mskB", [128, 128], f32, kind="ExternalInput")
    loss_d = nc.dram_tensor("loss", [128, TILES_PER_CORE], f32, kind="ExternalOutput")

    with ExitStack() as ctx:
        tc = ctx.enter_context(TileContext(nc))
        const = ctx.enter_context(tc.tile_pool(name="const", bufs=1))
        egp = ctx.enter_context(tc.tile_pool(name="egp", bufs=2))
        psum = ctx.enter_context(
            tc.tile_pool(name="psum", bufs=4, space=bass.MemorySpace.PSUM)
        )
        stats = ctx.enter_context(tc.tile_pool(name="stats", bufs=3))

        zqt = []
        for q in range(4):
            zt = const.tile([128, 2048], f16, tag=f"zq{q}")
            nc.gpsimd.dma_start(out=zt, in_=zq_d[q][:])
            zqt.append(zt)
        posit = const.tile([128, 128], f32, tag="posI")
        nc.gpsimd.dma_start(out=posit, in_=posi_d[:])
        mskbt = const.tile([128, 128], f32, tag="mskB")
        nc.gpsimd.dma_start(out=mskbt, in_=mskb_d[:])
        losst = const.tile([128, TILES_PER_CORE], f32, tag="losst")

        def chunk_matmuls(P, t, e):
            # chunk e covers global cols [e*CHUNK, (e+1)*CHUNK)
            lhsT = zqt[0][:, t * 128 : (t + 1) * 128]
            dj = (t * 128) // 512  # 512-piece of chunk 0 containing self-diag
            for j in range(2):
                gcol = e * CHUNK + j * 512
                is_diag_chunk = e == 0 and j == dj
                nc.tensor.matmul(
                    P[:, j * 512 : (j + 1) * 512],
                    lhsT,
                    zqt[gcol // 2048][:, gcol % 2048 : gcol % 2048 + 512],
                    start=True,
                    stop=not is_diag_chunk,
                    skip_group_check=True,
                )
                if is_diag_chunk:
                    # self-diag block += -1e5*I  (I.T @ (-1e5*I) accumulated)
                    nc.tensor.matmul(
                        P[:, t * 128 : t * 128 + 128],
                        posit,
                        mskbt,
                        start=False,
                        stop=True,
                        skip_group_check=True,
                    )

        POS_E = 4096 // CHUNK  # chunk holding the positive-pair diagonal
        for t in range(TILES_PER_CORE):
            nm = stats.tile([128, NCHUNK], f32, tag="nm")   # -m_e per chunk
            ssq = stats.tile([128, NCHUNK], f32, tag="ssq")  # sum e^{x-m_e}
            pos = stats.tile([128, 1], f32, tag="pos")
            scr = stats.tile([128, 128], f32, tag="scr")

            for e in range(NCHUNK):
                P = psum.tile([128, CHUNK], f32, tag="P")
                chunk_matmuls(P, t, e)
                if e == POS_E:
                    # positive-pair diag at chunk-local cols [t*128, +128)
                    nc.vector.scalar_tensor_tensor(
                        out=scr,
                        in0=P[:, t * 128 : t * 128 + 128],
                        scalar=1.0,
                        in1=posit,
                        op0=Alu.mult,
                        op1=Alu.mult,
                        accum_out=pos,
                    )
                nc.vector.reduce_max(out=nm[:, e : e + 1], in_=P, axis=X, negate=True)
                eg = egp.tile([128, CHUNK], f32, tag="eg")
                nc.scalar.activation(
                    out=eg,
                    in_=P,
                    func=Act.Exp,
                    bias=nm[:, e : e + 1],
                    scale=1.0,
                    accum_out=ssq[:, e : e + 1],
                )

            # tail: exact recombination of the four quarter-shifted sums
            # m = global row max = -min_q nm_q  (mt := -m)
            mt = stats.tile([128, 1], f32, tag="mt")
            nc.vector.tensor_reduce(out=mt, in_=nm, axis=X, op=Alu.min)
            f = stats.tile([128, NCHUNK], f32, tag="f")
            nc.scalar.activation(out=f, in_=nm, func=Act.Exp, scale=-1.0, bias=mt)
            s03 = stats.tile([128, 1], f32, tag="s03")
            nc.vector.scalar_tensor_tensor(
                out=scr[:, 0:NCHUNK],
                in0=ssq,
                scalar=1.0,
                in1=f,
                op0=Alu.mult,
                op1=Alu.mult,
                accum_out=s03,
            )
            pe = stats.tile([128, 1], f32, tag="pe")   # e^{pos - m}
            nc.scalar.activation(out=pe, in_=pos, func=Act.Exp, bias=mt, scale=1.0)
            stot = stats.tile([128, 1], f32, tag="stot")
            nc.vector.tensor_add(stot, s03, pe)
            # loss = ln(stot) + m - pos = lg - mt - pos
            lg = stats.tile([128, 1], f32, tag="lg")
            nc.scalar.activation(out=lg, in_=stot, func=Act.Ln)
            lp = stats.tile([128, 1], f32, tag="lp")
            nc.vector.tensor_sub(lp, lg, mt)
            nc.vector.tensor_sub(losst[:, t : t + 1], lp, pos)

        nc.gpsimd.dma_start(out=loss_d[:], in_=losst)

    _split_waits(nc)
    return nc


def _get_module():
    if "nc" not in _cached:
        _cached["nc"] = _build_module()
    return _cached["nc"]


def _host_inputs(z_i, z_j):
    z = np.concatenate(
        [np.asarray(z_i, np.float32), np.asarray(z_j, np.float32)], axis=0
    )
    s = np.float32(1.0 / np.sqrt(TEMP))
    zT = np.ascontiguousarray((z * s).T).astype(np.float16)  # [128, 8192]

    posI = np.eye(128, dtype=np.float32)
    mskB = np.float32(MASK_NEG) * np.eye(128, dtype=np.float32)

    in_maps = []
    for c in range(N_CORES):
        k = c * ROWS_PER_CORE
        rot = np.concatenate([zT[:, k:], zT[:, :k]], axis=1)
        im = {
            f"zq{q}": np.ascontiguousarray(rot[:, q * 2048 : (q + 1) * 2048])
            for q in range(4)
        }
        im["posI"] = posI
        im["mskB"] = mskB
        in_maps.append(im)
    return in_maps


def run_full(z_i, z_j, trace=False, trace_kwargs=None):
    """Run on 8 cores; returns (loss_scalar, BassKernelResults)."""
    from concourse.bass_utils import run_bass_kernel_spmd

    nc = _get_module()
    in_maps = _host_inputs(z_i, z_j)
    res = run_bass_kernel_spmd(
        nc,
        in_maps,
        core_ids=list(range(N_CORES)),
        trace=trace,
        **(trace_kwargs or {}),
    )
    total = np.float64(0.0)
    for c in range(N_CORES):
        total += res.results[c]["loss"].astype(np.float64).sum()
    loss = np.array(total / NROWS, dtype=np.float32)
    return loss, res


def kernel(z_i, z_j):
    loss, _ = run_full(z_i, z_j, trace=bool(os.environ.get("KERNEL_TRACE")))
    return loss


# revision 16
# speedup vs baseline: 1.0241x; 1.0084x over previous
"""NT-Xent loss (B=4096, D=128, T=0.07) on 8 Trainium2 NeuronCores.

Strategy (one SPMD Bass program, 8 cores):
  - Host: z = concat(z_i, z_j) [8192,128], scale by 1/sqrt(T), transpose to
    zT [128, 8192], cast fp16 (PE runs fp16 at 4x the fp32 rate; validated
    loss rel-err ~1.4e-6).  Core c receives zT rotated left by c*1024 cols so
    its own 1024 rows sit at columns 0..1023 -> the self-sim diag block and
    the positive-pair diag block land at compile-time-constant offsets on
    every core (one uniform SPMD program).
  - Device, per 128-row tile t (8 tiles/core), ONE pass over the [128, 8192]
    similarity slab in eight [128,1024] PSUM chunks (4 PSUM buffers in flight so the
    fill->reduce->exp->release chain pipelines):
      PE   : 2 matmuls (N=512, fp16) per chunk; the self-diag block gets
             -1e5*I added via an extra accumulating matmul (identity trick),
             so it can never win the max and exp() flushes it to 0.
      DVE  : reduce_max(negate) straight from PSUM -> per-chunk -max m_e;
             scalar_tensor_tensor extracts the positive-pair diagonal.
      ACT  : activation(Exp, bias=-m_e, accum_out) straight from the same
             PSUM chunk = fused exp + row-sum with a PER-CHUNK shift.
             (DVE and ACT read the same chunk concurrently via separate
             PSUM ports; PE fills the other buffers meanwhile.)
      tail : chunk sums are rescaled exactly: stot = sum_e ssq_e*e^{m_e-m}
             (+ e^{pos-m} for the duplicated positive), loss = ln(stot)+m-pos.
             All tail ops are [128,8]/[128,1] sized.
  - Host: sum the 8 x [128,8] per-row losses, divide by 8192.

This avoids any second PE pass and any PSUM->SBUF evacuation of the slab:
each PSUM element is read exactly twice (once by DVE for the max, once by
ACT for the exp-sum), which is the minimum this algorithm needs.

The toolchain's walrus allows only ONE sync-wait per TPB instruction;
_split_waits() hoists extra waits onto injected NoOps post-Tile.
"""

import os
import numpy as np

N_CORES = 8
B = 4096
NROWS = 2 * B           # 8192
ROWS_PER_CORE = NROWS // N_CORES       # 1024
TILES_PER_CORE = ROWS_PER_CORE // 128  # 8
CHUNK = 1024
NCHUNK = 8192 // CHUNK  # 8
TEMP = 0.07
MASK_NEG = -1.0e5

_cached = {}


def _split_waits(nc, limit=1):
    import bass_rust
    import concourse.mybir as mybir

    n = 0
    for f in nc.m.functions:
        for blk in f.blocks:
            new_insts = []
            for inst in blk.instructions:
                si = inst.sync_info
                waits = list(si.on_wait) if (si and si.on_wait) else []
                if len(waits) > limit:
                    for w in waits[:-limit]:
                        nop = bass_rust.InstNoOp(name=f"waitnop-{n}")
                        n += 1
                        nop.engine = inst.engine
                        nop.sync_info = mybir.SyncInfo(on_wait=[w], on_update=[])
                        new_insts.append(nop)
                    inst.sync_info = mybir.SyncInfo(
                        on_wait=waits[-limit:], on_update=list(si.on_update or [])
                    )
                new_insts.append(inst)
            blk.instructions = new_insts


def _build_module():
    import concourse.bass as bass
    import concourse.mybir as mybir
    from concourse.tile import TileContext
    from contextlib import ExitStack

    f32 = mybir.dt.float32
    f16 = mybir.dt.float16
    Alu = mybir.AluOpType
    Act = mybir.ActivationFunctionType
    X = mybir.AxisListType.X

    nc = bass.Bass()

    zq_d = [
        nc.dram_tensor(f"zq{q}", [128, 2048], f16, kind="ExternalInput")
        for q in range(4)
    ]
    posi_d = nc.dram_tensor("posI", [128, 128], f32, kind="ExternalInput")
    mskb_d = nc.dram_tensor("mskB", [128, 128], f32, kind="ExternalInput")
    loss_d = nc.dram_tensor("loss", [128, TILES_PER_CORE], f32, kind="ExternalOutput")

    with ExitStack() as ctx:
        tc = ctx.enter_context(TileContext(nc))
        const = ctx.enter_context(tc.tile_pool(name="const", bufs=1))
        egp = ctx.enter_context(tc.tile_pool(name="egp", bufs=2))
        psum = ctx.enter_context(
            tc.tile_pool(name="psum", bufs=4, space=bass.MemorySpace.PSUM)
        )
        stats = ctx.enter_context(tc.tile_pool(name="stats", bufs=3))

        zqt = []
        for q in range(4):
            zt = const.tile([128, 2048], f16, tag=f"zq{q}")
            nc.sync.dma_start(out=zt, in_=zq_d[q][:])
            zqt.append(zt)
        posit = const.tile([128, 128], f32, tag="posI")
        nc.sync.dma_start(out=posit, in_=posi_d[:])
        mskbt = const.tile([128, 128], f32, tag="mskB")
        nc.sync.dma_start(out=mskbt, in_=mskb_d[:])
        losst = const.tile([128, TILES_PER_CORE], f32, tag="losst")

        def chunk_matmuls(P, t, e):
            # chunk e covers global cols [e*CHUNK, (e+1)*CHUNK)
            lhsT = zqt[0][:, t * 128 : (t + 1) * 128]
            dj = (t * 128) // 512  # 512-piece of chunk 0 containing self-diag
            for j in range(2):
                gcol = e * CHUNK + j * 512
                is_diag_chunk = e == 0 and j == dj
                nc.tensor.matmul(
                    P[:, j * 512 : (j + 1) * 512],
                    lhsT,
                    zqt[gcol // 2048][:, gcol % 2048 : gcol % 2048 + 512],
                    start=True,
                    stop=not is_diag_chunk,
                    skip_group_check=True,
                )
                if is_diag_chunk:
                    # self-diag block += -1e5*I  (I.T @ (-1e5*I) accumulated)
                    nc.tensor.matmul(
                        P[:, t * 128 : t * 128 + 128],
                        posit,
                        mskbt,
                        start=False,
                        stop=True,
                        skip_group_check=True,
                    )

        POS_E = 4096 // CHUNK  # chunk holding the positive-pair diagonal
        for t in range(TILES_PER_CORE):
            nm = stats.tile([128, NCHUNK], f32, tag="nm")   # -m_e per chunk
            ssq = stats.tile([128, NCHUNK], f32, tag="ssq")  # sum e^{x-m_e}
            pos = stats.tile([128, 1], f32, tag="pos")
            scr = stats.tile([128, 128], f32, tag="scr")

            for e in range(NCHUNK):
                P = psum.tile([128, CHUNK], f32, tag="P")
                chunk_matmuls(P, t, e)
                if e == POS_E:
                    # positive-pair diag at chunk-local cols [t*128, +128)
                    nc.vector.scalar_tensor_tensor(
                        out=scr,
                        in0=P[:, t * 128 : t * 128 + 128],
                        scalar=1.0,
                        in1=posit,
                        op0=Alu.mult,
                        op1=Alu.mult,
                        accum_out=pos,
                    )
                nc.vector.reduce_max(out=nm[:, e : e + 1], in_=P, axis=X, negate=True)
                eg = egp.tile([128, CHUNK], f32, tag="eg")
                nc.scalar.activation(
                    out=eg,
                    in_=P,
                    func=Act.Exp,
                    bias=nm[:, e : e + 1],
                    scale=1.0,
                    accum_out=ssq[:, e : e + 1],
                )

            # tail: exact recombination of the four quarter-shifted sums
            # m = global row max = -min_q nm_q  (mt := -m)
            mt = stats.tile([128, 1], f32, tag="mt")
            nc.vector.tensor_reduce(out=mt, in_=nm, axis=X, op=Alu.min)
            f = stats.tile([128, NCHUNK], f32, tag="f")
            nc.scalar.activation(out=f, in_=nm, func=Act.Exp, scale=-1.0, bias=mt)
            s03 = stats.tile([128, 1], f32, tag="s03")
            nc.vector.scalar_tensor_tensor(
                out=scr[:, 0:NCHUNK],
                in0=ssq,
                scalar=1.0,
                in1=f,
                op0=Alu.mult,
                op1=Alu.mult,
                accum_out=s03,
            )
            pe = stats.tile([128, 1], f32, tag="pe")   # e^{pos - m}
            nc.scalar.activation(out=pe, in_=pos, func=Act.Exp, bias=mt, scale=1.0)
            stot = stats.tile([128, 1], f32, tag="stot")
            nc.vector.tensor_add(stot, s03, pe)
            # loss = ln(stot) + m - pos = lg - mt - pos
            lg = stats.tile([128, 1], f32, tag="lg")
            nc.scalar.activation(out=lg, in_=stot, func=Act.Ln)
            lp = stats.tile([128, 1], f32, tag="lp")
            nc.vector.tensor_sub(lp, lg, mt)
            nc.vector.tensor_sub(losst[:, t : t + 1], lp, pos)

        nc.gpsimd.dma_start(out=loss_d[:], in_=losst)

    _split_waits(nc)
    return nc


def _get_module():
    if "nc" not in _cached:
        _cached["nc"] = _build_module()
    return _cached["nc"]


def _host_inputs(z_i, z_j):
    z = np.concatenate(
        [np.asarray(z_i, np.float32), np.asarray(z_j, np.float32)], axis=0
    )
    s = np.float32(1.0 / np.sqrt(TEMP))
    zT = np.ascontiguousarray((z * s).T).astype(np.float16)  # [128, 8192]

    posI = np.eye(128, dtype=np.float32)
    mskB = np.float32(MASK_NEG) * np.eye(128, dtype=np.float32)

    in_maps = []
    for c in range(N_CORES):
        k = c * ROWS_PER_CORE
        rot = np.concatenate([zT[:, k:], zT[:, :k]], axis=1)
        im = {
            f"zq{q}": np.ascontiguousarray(rot[:, q * 2048 : (q + 1) * 2048])
            for q in range(4)
        }
        im["posI"] = posI
        im["mskB"] = mskB
        in_maps.append(im)
    return in_maps


def run_full(z_i, z_j, trace=False, trace_kwargs=None):
    """Run on 8 cores; returns (loss_scalar, BassKernelResults)."""
    from concourse.bass_utils import run_bass_kernel_spmd

    nc = _get_module()
    in_maps = _host_inputs(z_i, z_j)
    res = run_bass_kernel_spmd(
        nc,
        in_maps,
        core_ids=list(range(N_CORES)),
        trace=trace,
        **(trace_kwargs or {}),
    )
    total = np.float64(0.0)
    for c in range(N_CORES):
        total += res.results[c]["loss"].astype(np.float64).sum()
    loss = np.array(total / NROWS, dtype=np.float32)
    return loss, res


def kernel(z_i, z_j):
    loss, _ = run_full(z_i, z_j, trace=bool(os.environ.get("KERNEL_TRACE")))
    return loss
